# revision 28
# baseline (speedup 1.0000x reference)
"""BigBird-style block-sparse attention on 8 trn2 NeuronCores.

Sharding: core c -> (batch b = c//4) x (head-group g = c%4, 4 heads each).
Each core computes QKV for its (b, heads), block-sparse attention, and a
partial output (its heads' slice of the Wo contraction); the host sums the
4 partials per batch.

Layout strategy (all matmuls bf16, f32 PSUM accumulation):
  - hidden is pre-transposed on host to hiddenT [HID, S] bf16 so every
    projection has the contraction dim (HID) on partitions.
  - QT/KT [DH, S] per head; V [S, DH] natural, augmented with a ones
    column per head (V_aug [S, DH+1]).
  - scoresT tile = KT_tile.T @ QT_tile -> [k 128, q 128] in PSUM.
    exp on ACT (fuses the 1/sqrt(DH) scale; no max subtraction needed:
    |scores| <~ 5 for this data). Partial tiles multiplied by a cached
    0/1 bf16 mask pattern (only 19 distinct patterns exist).
  - AV: O[q, DH+1] += expT_tile.T @ V_aug  -- the ones column makes
    col DH the softmax denominator, for free.
  - normalize via ACT per-partition scale = 1/denom, PE-transpose the
    [q, DH] tile to OT [DH, q], which is exactly the lhsT layout the
    Wo matmul needs. Host sums partial outputs.
"""

import sys

sys.path.insert(0, "/opt/trn_rl_repo")

import numpy as np
import ml_dtypes

import concourse.bacc as bacc
import concourse.bass as bass
import concourse.mybir as mybir
import concourse.tile as tile
from concourse.bass_utils import run_bass_kernel_spmd

BF16 = ml_dtypes.bfloat16

B, S, HID = 2, 2048, 2048
H, DH = 16, 128
SCALE = DH**-0.5
NCORES = 8
GROUPS = 4          # head groups (tensor parallel)
HPG = H // GROUPS   # heads per group = 4
GW = HPG * DH       # group width = 512
NT = S // 128       # 16 q/k tiles of 128

_cache = {}


def _mask_meta(mask):
    """Per-(qtile, ktile) activity and dedup'd partial patterns.

    Returns (active, patterns) where active[t] = list of (u, pat_id),
    pat_id = -1 for fully-active tiles, and patterns is [Npat,128,128]
    float32 0/1 in TRANSPOSED [k, q] orientation.
    """
    pats = {}
    patterns = []
    active = []
    for t in range(NT):
        row = []
        for u in range(NT):
            tm = mask[t * 128 : (t + 1) * 128, u * 128 : (u + 1) * 128]
            if not tm.any():
                continue
            if tm.all():
                row.append((u, -1))
            else:
                key = tm.tobytes()
                if key not in pats:
                    pats[key] = len(patterns)
                    patterns.append(tm.T.astype(np.float32))
                row.append((u, pats[key]))
        active.append(row)
    return active, np.stack(patterns)


def _build(active, npat):
    """Build the single-core SPMD Bass program."""
    nc = bacc.Bacc(None, target_bir_lowering=False)
    f32 = mybir.dt.float32
    bf16 = mybir.dt.bfloat16

    hT_d = nc.dram_tensor("hiddenT", [HID, S], bf16, kind="ExternalInput")
    wq_d = nc.dram_tensor("wq", [HID, GW], bf16, kind="ExternalInput")
    wk_d = nc.dram_tensor("wk", [HID, GW], bf16, kind="ExternalInput")
    wv_d = nc.dram_tensor("wv", [HID, GW], bf16, kind="ExternalInput")
    wo_d = nc.dram_tensor("wo", [GW, HID], bf16, kind="ExternalInput")
    mp_d = nc.dram_tensor("maskpat", [npat * 128, 128], bf16, kind="ExternalInput")
    id_d = nc.dram_tensor("ident", [128, 128], bf16, kind="ExternalInput")
    out_d = nc.dram_tensor("out", [S, HID], bf16, kind="ExternalOutput")

    NK = HID // 128  # 16 contraction tiles
    VW = GW + HPG  # 516: V_aug row width (4 heads x 129)

    with tile.TileContext(nc) as tc:
        with (
            tc.tile_pool(name="persist", bufs=1) as pp,
            tc.tile_pool(name="qk", bufs=3) as qkp,
            tc.tile_pool(name="exp_pool", bufs=4) as ep,
            tc.tile_pool(name="small", bufs=6) as sp,
            tc.tile_pool(name="psum_s", bufs=3, space="PSUM") as pss,
            tc.tile_pool(name="psum_o", bufs=2, space="PSUM") as pso,
            tc.tile_pool(name="psum_t", bufs=1, space="PSUM") as pst,
        ):
            # Persistent SBUF arrays (live across phases).
            v_all = pp.tile([128, NT * VW], bf16)   # V_aug per ktile: 4x129
            ot_all = pp.tile([128, HPG * S], bf16)  # OT per (head, qtile)
            ident = pp.tile([128, 128], bf16)
            masks = pp.tile([128, npat * 128], bf16)
            nc.sync.dma_start(ident[:], id_d[:])
            nc.sync.dma_start(
                masks[:].rearrange("p (k n) -> p k n", n=128),
                mp_d[:].rearrange("(k p) n -> p k n", p=128),
            )

            # Interleaved QKV projections + per-head attention: V first,
            # then per head (QT,KT then that head's attention) so the
            # scheduler overlaps attention h with projections h+1.
            mul_flip = [0]

            with (
                tc.tile_pool(name="phb", bufs=1) as pb,
                tc.tile_pool(name="psum_proj", bufs=2, space="PSUM") as psb,
            ):
                hT = pb.tile([128, NK * S], bf16)
                wq = pb.tile([128, NK * GW], bf16)
                wk = pb.tile([128, NK * GW], bf16)
                wv = pb.tile([128, NK * GW], bf16)
                # Few, large, multi-tile DMAs: HWDGE dispatch is ~0.6us per
                # dma_start and serializes on the Sync sequencer; each DMA
                # fans out across the 16 SDMA queues by shape. Group rows
                # [k*128+p, :] -> sbuf [p, k, :] with a 3D access pattern,
                # interleaved so early kt-tiles land first.
                def gload(dst_sb, src_d, k0, nk_, w):
                    src = src_d[k0 * 128 : (k0 + nk_) * 128, :].rearrange(
                        "(k p) s -> p k s", p=128
                    )
                    dst = dst_sb[:, k0 * w : (k0 + nk_) * w].rearrange(
                        "p (k s) -> p k s", s=w
                    )
                    nc.sync.dma_start(dst, src)

                # hT loaded in COLUMN-groups: V chain st and QK chunk qc
                # consume hT columns in order, so after wv + columns 0:512
                # of all 16 kt (~4MB) whole chains run at full speed.
                def gcload(k0, nk_, c0, cw):
                    src = hT_d[k0 * 128 : (k0 + nk_) * 128, c0 : c0 + cw].rearrange(
                        "(k p) s -> p k s", p=128
                    )
                    dst = hT[:, k0 * S : (k0 + nk_) * S].rearrange(
                        "p (k s) -> p k s", s=S
                    )[:, :, c0 : c0 + cw]
                    nc.sync.dma_start(dst, src)

                gload(wv, wv_d, 0, 1, GW)
                gcload(0, 1, 0, 512)
                gload(wv, wv_d, 1, 3, GW)
                gcload(1, 3, 0, 512)
                gload(wv, wv_d, 4, 4, GW)
                gcload(4, 4, 0, 512)
                gload(wv, wv_d, 8, 8, GW)
                gcload(8, 8, 0, 512)
                gload(wq, wq_d, 0, 4, GW)
                gload(wk, wk_d, 0, 4, GW)
                gcload(0, 8, 512, 512)
                gload(wq, wq_d, 4, 4, GW)
                gload(wk, wk_d, 4, 4, GW)
                gcload(8, 8, 512, 512)
                gload(wq, wq_d, 8, 8, GW)
                gload(wk, wk_d, 8, 8, GW)
                gcload(0, 8, 1024, 512)
                gcload(8, 8, 1024, 512)
                gcload(0, 8, 1536, 512)
                gcload(8, 8, 1536, 512)

                nc.any.memset(v_all[:], 1.0)


                # V: per s-tile, contract over kt; scatter heads into V_aug.
                for st in range(NT):
                    ps = psb.tile([128, 512], f32, tag="proj")
                    for kt in range(NK):
                        nc.tensor.matmul(
                            ps[:],
                            hT[:, kt * S + st * 128 : kt * S + (st + 1) * 128],
                            wv[:, kt * GW : (kt + 1) * GW],
                            start=(kt == 0),
                            stop=(kt == NK - 1),
                        )
                    nc.vector.tensor_copy(
                        v_all[:, st * VW : (st + 1) * VW].rearrange(
                            "p (h d) -> p h d", d=DH + 1
                        )[:, :, 0:DH],
                        ps[:].rearrange("p (h d) -> p h d", d=DH),
                    )

                for h in range(HPG):
                    # QT / KT for this head.
                    qt = qkp.tile([128, S], bf16, tag="qt")
                    ktt = qkp.tile([128, S], bf16, tag="kt")
                    for w_sb, o_sb in ((wq, qt), (wk, ktt)):
                        for qc in range(S // 512):
                            ps = psb.tile([128, 512], f32, tag="proj")
                            for kt in range(NK):
                                nc.tensor.matmul(
                                    ps[:],
                                    w_sb[:, kt * GW + h * DH : kt * GW + (h + 1) * DH],
                                    hT[:, kt * S + qc * 512 : kt * S + (qc + 1) * 512],
                                    start=(kt == 0),
                                    stop=(kt == NK - 1),
                                )
                            nc.scalar.copy(
                                o_sb[:, qc * 512 : (qc + 1) * 512], ps[:]
                            )

                    # Attention for this head, 4 k-tiles batched per exp.
                    for t in range(NT):
                        ops = pso.tile([128, DH + 1], f32, tag="av")
                        acts = active[t]
                        n_act = len(acts)
                        j = 0
                        for c0 in range(0, n_act, 4):
                            chunk = acts[c0 : c0 + 4]
                            n = len(chunk)
                            sps = pss.tile([128, 512], f32, tag="sc")
                            for i, (u, pid) in enumerate(chunk):
                                nc.tensor.matmul(
                                    sps[:, i * 128 : (i + 1) * 128],
                                    ktt[:, u * 128 : (u + 1) * 128],
                                    qt[:, t * 128 : (t + 1) * 128],
                                    start=True,
                                    stop=True,
                                )
                            e_sb = ep.tile([128, 512], bf16, tag="exp")
                            nc.scalar.activation(
                                e_sb[:, : n * 128],
                                sps[:, : n * 128],
                                mybir.ActivationFunctionType.Exp,
                                scale=SCALE,
                            )
                            for i, (u, pid) in enumerate(chunk):
                                if pid >= 0:
                                    eng = (
                                        nc.vector if mul_flip[0] % 2 else nc.gpsimd
                                    )
                                    mul_flip[0] += 1
                                    eng.tensor_mul(
                                        e_sb[:, i * 128 : (i + 1) * 128],
                                        e_sb[:, i * 128 : (i + 1) * 128],
                                        masks[:, pid * 128 : (pid + 1) * 128],
                                    )
                                nc.tensor.matmul(
                                    ops[:],
                                    e_sb[:, i * 128 : (i + 1) * 128],
                                    v_all[
                                        :,
                                        u * VW
                                        + h * (DH + 1) : u * VW
                                        + (h + 1) * (DH + 1),
                                    ],
                                    start=(j == 0),
                                    stop=(j == n_act - 1),
                                )
                                j += 1
                        rc = sp.tile([128, 1], f32, tag="rc")
                        nc.vector.reciprocal(rc[:], ops[:, DH : DH + 1])
                        on = sp.tile([128, 128], bf16, tag="onorm")
                        nc.vector.tensor_scalar_mul(on[:], ops[:, 0:DH], rc[:])
                        tp = pst.tile([128, 128], bf16, tag="tr")
                        nc.tensor.transpose(tp[:], on[:], ident[:])
                        nc.scalar.copy(
                            ot_all[:, h * S + t * 128 : h * S + (t + 1) * 128], tp[:]
                        )

            # ---------------- Phase D: output projection ----------------
            with (
                tc.tile_pool(name="phd", bufs=1) as pdp,
                tc.tile_pool(name="ostage", bufs=3) as osp,
                tc.tile_pool(name="psum_d", bufs=2, space="PSUM") as psd,
            ):
                wo = pdp.tile([128, HPG * HID], bf16)
                for h in range(HPG):
                    nc.sync.dma_start(
                        wo[:, h * HID : (h + 1) * HID],
                        wo_d[h * 128 : (h + 1) * 128, :],
                    )
                for t in range(NT):
                    for c in range(HID // 512):
                        ps = psd.tile([128, 512], f32, tag="out")
                        for h in range(HPG):
                            nc.tensor.matmul(
                                ps[:],
                                ot_all[:, h * S + t * 128 : h * S + (t + 1) * 128],
                                wo[:, h * HID + c * 512 : h * HID + (c + 1) * 512],
                                start=(h == 0),
                                stop=(h == HPG - 1),
                            )
                        ob = osp.tile([128, 512], bf16, tag="ostage")
                        nc.vector.tensor_copy(ob[:], ps[:])
                        nc.sync.dma_start(
                            out_d[t * 128 : (t + 1) * 128, c * 512 : (c + 1) * 512],
                            ob[:],
                        )

    nc.compile()
    return nc


def kernel(hidden_states, Wq, Wk, Wv, Wo, sparse_mask):
    hidden_states = np.asarray(hidden_states, np.float32)
    Wq = np.asarray(Wq, np.float32)
    Wk = np.asarray(Wk, np.float32)
    Wv = np.asarray(Wv, np.float32)
    Wo = np.asarray(Wo, np.float32)
    mask = np.asarray(sparse_mask, bool)

    key = mask.tobytes()
    if key not in _cache:
        active, patterns = _mask_meta(mask)
        nc = _build(active, len(patterns))
        _cache[key] = (nc, patterns)
    nc, patterns = _cache[key]

    mp = np.ascontiguousarray(
        patterns.reshape(len(patterns) * 128, 128).astype(BF16)
    )
    ident = np.eye(128, dtype=BF16)
    hT = [np.ascontiguousarray(hidden_states[b].T).astype(BF16) for b in range(B)]
    wq_g = [np.ascontiguousarray(Wq[:, g * GW : (g + 1) * GW]).astype(BF16) for g in range(GROUPS)]
    wk_g = [np.ascontiguousarray(Wk[:, g * GW : (g + 1) * GW]).astype(BF16) for g in range(GROUPS)]
    wv_g = [np.ascontiguousarray(Wv[:, g * GW : (g + 1) * GW]).astype(BF16) for g in range(GROUPS)]
    wo_g = [np.ascontiguousarray(Wo[g * GW : (g + 1) * GW, :]).astype(BF16) for g in range(GROUPS)]

    in_maps = []
    for c in range(NCORES):
        b, g = c // GROUPS, c % GROUPS
        in_maps.append(
            {
                "hiddenT": hT[b],
                "wq": wq_g[g],
                "wk": wk_g[g],
                "wv": wv_g[g],
                "wo": wo_g[g],
                "maskpat": mp,
                "ident": ident,
            }
        )

    res = run_bass_kernel_spmd(nc, in_maps, list(range(NCORES)))

    out = np.zeros((B, S, HID), np.float32)
    for c in range(NCORES):
        out[c // GROUPS] += np.asarray(res.results[c]["out"], np.float32)
    return out


# revision 30
# speedup vs baseline: 1.0248x; 1.0248x over previous
"""BigBird-style block-sparse attention on 8 trn2 NeuronCores.

Sharding: core c -> (batch b = c//4) x (head-group g = c%4, 4 heads each).
Each core computes QKV for its (b, heads), block-sparse attention, and a
partial output (its heads' slice of the Wo contraction); the host sums the
4 partials per batch.

Layout strategy (all matmuls bf16, f32 PSUM accumulation):
  - hidden is pre-transposed on host to hiddenT [HID, S] bf16 so every
    projection has the contraction dim (HID) on partitions.
  - QT/KT [DH, S] per head; V [S, DH] natural, augmented with a ones
    column per head (V_aug [S, DH+1]).
  - scoresT tile = KT_tile.T @ QT_tile -> [k 128, q 128] in PSUM.
    exp on ACT (fuses the 1/sqrt(DH) scale; no max subtraction needed:
    |scores| <~ 5 for this data). Partial tiles multiplied by a cached
    0/1 bf16 mask pattern (only 19 distinct patterns exist).
  - AV: O[q, DH+1] += expT_tile.T @ V_aug  -- the ones column makes
    col DH the softmax denominator, for free.
  - normalize via ACT per-partition scale = 1/denom, PE-transpose the
    [q, DH] tile to OT [DH, q], which is exactly the lhsT layout the
    Wo matmul needs. Host sums partial outputs.
"""

import sys

sys.path.insert(0, "/opt/trn_rl_repo")

import numpy as np
import ml_dtypes

import concourse.bacc as bacc
import concourse.bass as bass
import concourse.mybir as mybir
import concourse.tile as tile
from concourse.bass_utils import run_bass_kernel_spmd

BF16 = ml_dtypes.bfloat16

B, S, HID = 2, 2048, 2048
H, DH = 16, 128
SCALE = DH**-0.5
NCORES = 8
GROUPS = 4          # head groups (tensor parallel)
HPG = H // GROUPS   # heads per group = 4
GW = HPG * DH       # group width = 512
NT = S // 128       # 16 q/k tiles of 128

_cache = {}


def _mask_meta(mask):
    """Per-(qtile, ktile) activity and dedup'd partial patterns.

    Returns (active, patterns) where active[t] = list of (u, pat_id),
    pat_id = -1 for fully-active tiles, and patterns is [Npat,128,128]
    float32 0/1 in TRANSPOSED [k, q] orientation.
    """
    pats = {}
    patterns = []
    active = []
    for t in range(NT):
        row = []
        for u in range(NT):
            tm = mask[t * 128 : (t + 1) * 128, u * 128 : (u + 1) * 128]
            if not tm.any():
                continue
            if tm.all():
                row.append((u, -1))
            else:
                key = tm.tobytes()
                if key not in pats:
                    pats[key] = len(patterns)
                    patterns.append(tm.T.astype(np.float32))
                row.append((u, pats[key]))
        active.append(row)
    return active, np.stack(patterns)


def _build(active, npat):
    """Build the single-core SPMD Bass program."""
    nc = bacc.Bacc(None, target_bir_lowering=False)
    f32 = mybir.dt.float32
    bf16 = mybir.dt.bfloat16

    hT_d = nc.dram_tensor("hiddenT", [HID, S], bf16, kind="ExternalInput")
    wq_d = nc.dram_tensor("wq", [HID, GW], bf16, kind="ExternalInput")
    wk_d = nc.dram_tensor("wk", [HID, GW], bf16, kind="ExternalInput")
    wv_d = nc.dram_tensor("wv", [HID, GW], bf16, kind="ExternalInput")
    wo_d = nc.dram_tensor("wo", [GW, HID], bf16, kind="ExternalInput")
    mp_d = nc.dram_tensor("maskpat", [npat * 128, 128], bf16, kind="ExternalInput")
    id_d = nc.dram_tensor("ident", [128, 128], bf16, kind="ExternalInput")
    out_d = nc.dram_tensor("out", [S, HID], bf16, kind="ExternalOutput")

    NK = HID // 128  # 16 contraction tiles
    VW = GW + HPG  # 516: V_aug row width (4 heads x 129)

    with tile.TileContext(nc) as tc:
        with (
            tc.tile_pool(name="persist", bufs=1) as pp,
            tc.tile_pool(name="qk", bufs=4) as qkp,
            tc.tile_pool(name="exp_pool", bufs=4) as ep,
            tc.tile_pool(name="small", bufs=6) as sp,
            tc.tile_pool(name="psum_s", bufs=2, space="PSUM") as pss,
            tc.tile_pool(name="psum_o", bufs=2, space="PSUM") as pso,
            tc.tile_pool(name="psum_t", bufs=1, space="PSUM") as pst,
        ):
            # Persistent SBUF arrays (live across phases).
            v_all = pp.tile([128, NT * VW], bf16)   # V_aug per ktile: 4x129
            ot_all = pp.tile([128, HPG * S], bf16)  # OT per (head, qtile)
            ident = pp.tile([128, 128], bf16)
            masks = pp.tile([128, npat * 128], bf16)
            nc.sync.dma_start(ident[:], id_d[:])
            nc.sync.dma_start(
                masks[:].rearrange("p (k n) -> p k n", n=128),
                mp_d[:].rearrange("(k p) n -> p k n", p=128),
            )

            # Interleaved QKV projections + per-head attention: V first,
            # then per head (QT,KT then that head's attention) so the
            # scheduler overlaps attention h with projections h+1.
            mul_flip = [0]

            with (
                tc.tile_pool(name="phb", bufs=1) as pb,
                tc.tile_pool(name="psum_proj", bufs=3, space="PSUM") as psb,
            ):
                hT = pb.tile([128, NK * S], bf16)
                wq = pb.tile([128, NK * GW], bf16)
                wk = pb.tile([128, NK * GW], bf16)
                wv = pb.tile([128, NK * GW], bf16)
                # Few, large, multi-tile DMAs: HWDGE dispatch is ~0.6us per
                # dma_start and serializes on the Sync sequencer; each DMA
                # fans out across the 16 SDMA queues by shape. Group rows
                # [k*128+p, :] -> sbuf [p, k, :] with a 3D access pattern,
                # interleaved so early kt-tiles land first.
                def gload(dst_sb, src_d, k0, nk_, w):
                    src = src_d[k0 * 128 : (k0 + nk_) * 128, :].rearrange(
                        "(k p) s -> p k s", p=128
                    )
                    dst = dst_sb[:, k0 * w : (k0 + nk_) * w].rearrange(
                        "p (k s) -> p k s", s=w
                    )
                    nc.sync.dma_start(dst, src)

                # hT loaded in COLUMN-groups: V chain st and QK chunk qc
                # consume hT columns in order, so after wv + columns 0:512
                # of all 16 kt (~4MB) whole chains run at full speed.
                def gcload(k0, nk_, c0, cw):
                    src = hT_d[k0 * 128 : (k0 + nk_) * 128, c0 : c0 + cw].rearrange(
                        "(k p) s -> p k s", p=128
                    )
                    dst = hT[:, k0 * S : (k0 + nk_) * S].rearrange(
                        "p (k s) -> p k s", s=S
                    )[:, :, c0 : c0 + cw]
                    nc.sync.dma_start(dst, src)

                gload(wv, wv_d, 0, 1, GW)
                gcload(0, 1, 0, 512)
                gload(wv, wv_d, 1, 3, GW)
                gcload(1, 3, 0, 512)
                gload(wv, wv_d, 4, 4, GW)
                gcload(4, 4, 0, 512)
                gload(wv, wv_d, 8, 8, GW)
                gcload(8, 8, 0, 512)
                gload(wq, wq_d, 0, 4, GW)
                gload(wk, wk_d, 0, 4, GW)
                gcload(0, 8, 512, 512)
                gload(wq, wq_d, 4, 4, GW)
                gload(wk, wk_d, 4, 4, GW)
                gcload(8, 8, 512, 512)
                gload(wq, wq_d, 8, 8, GW)
                gload(wk, wk_d, 8, 8, GW)
                gcload(0, 8, 1024, 512)
                gcload(8, 8, 1024, 512)
                gcload(0, 8, 1536, 512)
                gcload(8, 8, 1536, 512)

                nc.any.memset(v_all[:], 1.0)


                # V: per s-tile, contract over kt; scatter heads into V_aug.
                for st in range(NT):
                    ps = psb.tile([128, 512], f32, tag="proj")
                    for kt in range(NK):
                        nc.tensor.matmul(
                            ps[:],
                            hT[:, kt * S + st * 128 : kt * S + (st + 1) * 128],
                            wv[:, kt * GW : (kt + 1) * GW],
                            start=(kt == 0),
                            stop=(kt == NK - 1),
                        )
                    nc.vector.tensor_copy(
                        v_all[:, st * VW : (st + 1) * VW].rearrange(
                            "p (h d) -> p h d", d=DH + 1
                        )[:, :, 0:DH],
                        ps[:].rearrange("p (h d) -> p h d", d=DH),
                    )

                qts = []
                for h in range(HPG):
                    # QT / KT for this head.
                    qt = qkp.tile([128, S], bf16, tag="qt")
                    ktt = qkp.tile([128, S], bf16, tag="kt")
                    qts.append((qt, ktt))
                    for w_sb, o_sb in ((wq, qt), (wk, ktt)):
                        for qc in range(S // 512):
                            ps = psb.tile([128, 512], f32, tag="proj")
                            for kt in range(NK):
                                nc.tensor.matmul(
                                    ps[:],
                                    w_sb[:, kt * GW + h * DH : kt * GW + (h + 1) * DH],
                                    hT[:, kt * S + qc * 512 : kt * S + (qc + 1) * 512],
                                    start=(kt == 0),
                                    stop=(kt == NK - 1),
                                )
                            nc.scalar.copy(
                                o_sb[:, qc * 512 : (qc + 1) * 512], ps[:]
                            )

            # ---- Attention (t-outer, h-inner) fused with per-tile Wo ----
            with (
                tc.tile_pool(name="phd", bufs=1) as pdp,
                tc.tile_pool(name="ostage", bufs=3) as osp,
                tc.tile_pool(name="psum_d", bufs=3, space="PSUM") as psd,
            ):
                wo = pdp.tile([128, HPG * HID], bf16)
                for h in range(HPG):
                    nc.sync.dma_start(
                        wo[:, h * HID : (h + 1) * HID],
                        wo_d[h * 128 : (h + 1) * 128, :],
                    )
                for t in range(NT):
                    acts = active[t]
                    n_act = len(acts)
                    for h in range(HPG):
                        qt, ktt = qts[h]
                        ops = pso.tile([128, DH + 1], f32, tag="av")
                        j = 0
                        for c0 in range(0, n_act, 4):
                            chunk = acts[c0 : c0 + 4]
                            n = len(chunk)
                            sps = pss.tile([128, 512], f32, tag="sc")
                            for i, (u, pid) in enumerate(chunk):
                                nc.tensor.matmul(
                                    sps[:, i * 128 : (i + 1) * 128],
                                    ktt[:, u * 128 : (u + 1) * 128],
                                    qt[:, t * 128 : (t + 1) * 128],
                                    start=True,
                                    stop=True,
                                )
                            e_sb = ep.tile([128, 512], bf16, tag="exp")
                            nc.scalar.activation(
                                e_sb[:, : n * 128],
                                sps[:, : n * 128],
                                mybir.ActivationFunctionType.Exp,
                                scale=SCALE,
                            )
                            for i, (u, pid) in enumerate(chunk):
                                if pid >= 0:
                                    eng = (
                                        nc.vector if mul_flip[0] % 2 else nc.gpsimd
                                    )
                                    mul_flip[0] += 1
                                    eng.tensor_mul(
                                        e_sb[:, i * 128 : (i + 1) * 128],
                                        e_sb[:, i * 128 : (i + 1) * 128],
                                        masks[:, pid * 128 : (pid + 1) * 128],
                                    )
                                nc.tensor.matmul(
                                    ops[:],
                                    e_sb[:, i * 128 : (i + 1) * 128],
                                    v_all[
                                        :,
                                        u * VW
                                        + h * (DH + 1) : u * VW
                                        + (h + 1) * (DH + 1),
                                    ],
                                    start=(j == 0),
                                    stop=(j == n_act - 1),
                                )
                                j += 1
                        rc = sp.tile([128, 1], f32, tag="rc")
                        nc.vector.reciprocal(rc[:], ops[:, DH : DH + 1])
                        on = sp.tile([128, 128], bf16, tag="onorm")
                        nc.vector.tensor_scalar_mul(on[:], ops[:, 0:DH], rc[:])
                        tp = pst.tile([128, 128], bf16, tag="tr")
                        nc.tensor.transpose(tp[:], on[:], ident[:])
                        nc.scalar.copy(
                            ot_all[:, h * S + t * 128 : h * S + (t + 1) * 128], tp[:]
                        )
                    # Wo for this qtile, right after its last head.
                    for c in range(HID // 512):
                        ps = psd.tile([128, 512], f32, tag="out")
                        for h in range(HPG):
                            nc.tensor.matmul(
                                ps[:],
                                ot_all[:, h * S + t * 128 : h * S + (t + 1) * 128],
                                wo[:, h * HID + c * 512 : h * HID + (c + 1) * 512],
                                start=(h == 0),
                                stop=(h == HPG - 1),
                            )
                        ob = osp.tile([128, 512], bf16, tag="ostage")
                        nc.vector.tensor_copy(ob[:], ps[:])
                        nc.sync.dma_start(
                            out_d[t * 128 : (t + 1) * 128, c * 512 : (c + 1) * 512],
                            ob[:],
                        )

    nc.compile()
    return nc


def kernel(hidden_states, Wq, Wk, Wv, Wo, sparse_mask):
    hidden_states = np.asarray(hidden_states, np.float32)
    Wq = np.asarray(Wq, np.float32)
    Wk = np.asarray(Wk, np.float32)
    Wv = np.asarray(Wv, np.float32)
    Wo = np.asarray(Wo, np.float32)
    mask = np.asarray(sparse_mask, bool)

    key = mask.tobytes()
    if key not in _cache:
        active, patterns = _mask_meta(mask)
        nc = _build(active, len(patterns))
        _cache[key] = (nc, patterns)
    nc, patterns = _cache[key]

    mp = np.ascontiguousarray(
        patterns.reshape(len(patterns) * 128, 128).astype(BF16)
    )
    ident = np.eye(128, dtype=BF16)
    hT = [np.ascontiguousarray(hidden_states[b].T).astype(BF16) for b in range(B)]
    wq_g = [np.ascontiguousarray(Wq[:, g * GW : (g + 1) * GW]).astype(BF16) for g in range(GROUPS)]
    wk_g = [np.ascontiguousarray(Wk[:, g * GW : (g + 1) * GW]).astype(BF16) for g in range(GROUPS)]
    wv_g = [np.ascontiguousarray(Wv[:, g * GW : (g + 1) * GW]).astype(BF16) for g in range(GROUPS)]
    wo_g = [np.ascontiguousarray(Wo[g * GW : (g + 1) * GW, :]).astype(BF16) for g in range(GROUPS)]

    in_maps = []
    for c in range(NCORES):
        b, g = c // GROUPS, c % GROUPS
        in_maps.append(
            {
                "hiddenT": hT[b],
                "wq": wq_g[g],
                "wk": wk_g[g],
                "wv": wv_g[g],
                "wo": wo_g[g],
                "maskpat": mp,
                "ident": ident,
            }
        )

    res = run_bass_kernel_spmd(nc, in_maps, list(range(NCORES)))

    out = np.zeros((B, S, HID), np.float32)
    for c in range(NCORES):
        out[c // GROUPS] += np.asarray(res.results[c]["out"], np.float32)
    return out
